# revision 24
# baseline (speedup 1.0000x reference)
"""CGRUCell Trainium2 kernel — hybrid data-parallel x4 (batch) x
tensor-parallel x2 (units) on 8 NeuronCores, with the 3-multiply
Karatsuba/Gauss complex matmul.

The reference's [[Wr,-Wi],[Wi,Wr]] cat is the real form of
(X1 + iX2) @ (Wr - i Wi), i.e. effective W2 = -Wi. Per gate we
accumulate three psum banks over the 4096-long contraction:
  t1 = X1@K1 + H1@R1
  t2 = X2@K2 + H2@R2        (K2 = -imag kernel, host-negated)
  t3 = (X1+X2)@(K1+K2) + (H1+H2)@(R1+R2)
and combine re = t1 - t2, im = t3 - t1 - t2 on the vector engine:
9.7e9 MACs/core instead of 12.9e9 (0.75x; the fp32 roofline was the
baseline's limit). Matmuls run in bf16 (rel err ~5e-3 vs 2e-2 budget):
Karatsuba needs 1.5x the weight bytes and fp32 weights would be
HBM-bound. Only K1,K2,R1,R2 are loaded (50 MB/core); the sum matrices
K12,R12 are formed on-chip by Pool-engine adds.

DMA instruction count is minimized (HWDGE costs ~625ns per DMA,
serialized): the 4 weight matrices are host-packed into one [2048, 4,
3072] tensor and loaded 2 k-chunks x 2 matrices per DMA; the 7
K-major activation tensors are host-packed into one [7,128,16,256]
tensor loaded in 4 chunked DMAs.

Gate r is computed output-transposed (weights stationary) so r*h is
produced directly K-major for gate h's contraction. PSUM start=True
resets has_written for the whole bank, so every accumulation region
gets its own bank: gate r runs two cc-pair phases per column group.
re/im/sum12 of r*h are exchanged with the paired core via AllGather,
hidden under gate z. Gate z's weights are host-prescaled by 0.2 (the
hard-sigmoid slope); gate h evicts through ScalarE tanh and fuses the
final h_new = hh + z*(h - hh) + store.
"""

import sys

for _p in ("/opt/trn_rl_repo", "/root/.axon_site/_ro/trn_rl_repo"):
    if _p not in sys.path:
        sys.path.append(_p)

import numpy as np
import ml_dtypes

import concourse.bass as bass
import concourse.mybir as mybir
import concourse.tile as tile
from concourse import bacc
from concourse.bass_utils import run_bass_kernel_spmd

P = 128
U = 2048           # UNITS
B = 1024
N_CORES = 8
GROUPS = 4
BC = B // GROUPS   # 256 batch rows per core
MSUB = BC // P     # 2 m-subtiles
UC = U // 2        # 1024 complex cols per core
KCH = U // P       # 16 k-chunks per 2048-long half-contraction
KG = 2             # k-chunks loaded per weight DMA
F32 = mybir.dt.float32
BF = mybir.dt.bfloat16
BLK = 512
BF_NP = np.dtype(ml_dtypes.bfloat16)
ACT_NAMES = ("x1", "x2", "x12", "h1", "h2", "h12", "hTo")

_CACHE = {}


def _build_nc(repeat=1, sim_collective=False):
    nc = bacc.Bacc(None, target_bir_lowering=False)

    # packed K-major bf16 activations [part, kchunk, name*batch flat]
    ACT = nc.dram_tensor("ACT", [P, KCH, len(ACT_NAMES) * BC], BF,
                         kind="ExternalInput")
    # batch-major h_tm1, own columns (re|im): [256, 2048] f32
    hbm = nc.dram_tensor("hbm", [BC, 2 * UC], F32, kind="ExternalInput")
    # packed bf16 weights, (mat k1|k2|r1|r2, col) flattened per block so
    # each load is a 3-dim AP; z block pre-scaled 0.2; k2/r2 = -imag
    WZH = nc.dram_tensor("WZH", [U, 3, 2, 4 * BLK], BF,
                         kind="ExternalInput")
    WR = nc.dram_tensor("WR", [U, 2, 2, 4 * 2 * P], BF,
                        kind="ExternalInput")
    # [z'(re|im) | h(re|im)], z' = 0.2*b + 0.5
    bias = nc.dram_tensor("bias", [2 * 2 * UC], F32, kind="ExternalInput")
    # gate-r column-major bias, pre-scaled 0.2*b + 0.5: [128, 16]
    biasr = nc.dram_tensor("biasr", [P, KCH], F32, kind="ExternalInput")
    out = nc.dram_tensor("out", [BC, 2 * UC], F32, kind="ExternalOutput")

    WZH_r = WZH.rearrange("(kg kc p) g hb mc -> p kg kc g hb mc",
                          kc=KG, p=P)
    WR_r = WR.rearrange("(kg kc p) q ph mc -> p kg kc q ph mc",
                        kc=KG, p=P)
    hbm_r = hbm.rearrange("(m p) c -> p m c", p=P)
    out_r = out.rearrange("(m p) c -> p m c", p=P)

    with tile.TileContext(nc) as tc:
        with (
            tc.tile_pool(name="acts", bufs=1) as acts,
            tc.tile_pool(name="wpool", bufs=2) as wpool,
            tc.tile_pool(name="spool", bufs=3) as spool,
            tc.tile_pool(name="psum", bufs=1, space="PSUM") as psum,
            tc.tile_pool(name="small", bufs=2) as small,
            tc.tile_pool(name="bigs", bufs=1) as bigs,
            tc.tile_pool(name="dram", bufs=1, space="DRAM") as dram,
        ):
            A7 = acts.tile([P, KCH, len(ACT_NAMES) * BC], BF, tag="A7",
                           name="A7")
            s = {n: A7[:, :, i * BC : (i + 1) * BC]
                 for i, n in enumerate(ACT_NAMES)}
            # chunked loads so gate-r matmuls start after the first
            # (small) slice; hbmt/brcol are deferred below the gate-r
            # issue so they don't delay the first weight tiles
            for sl in (slice(0, 2), slice(2, 8), slice(8, 16)):
                nc.sync.dma_start(A7[:, sl, :], ACT[:, sl, :])

            brcol = small.tile([P, KCH], F32, tag="brcol", name="brcol",
                               bufs=1)
            nc.sync.dma_start(brcol[:], biasr[:])
            hbmt = bigs.tile([P, MSUB, 2 * UC], F32, tag="hbmt", name="hbmt")
            z_sb = bigs.tile([P, MSUB, 2 * UC], BF, tag="z_sb", name="z_sb")

            def wgroup(rep, key, srcview, kg, nmats, width):
                """One DMA loading [128, KG kchunks, nmats*width flat]
                of packed weights; access mats via flat col offsets."""
                t = wpool.tile([P, KG, nmats * width], BF,
                               tag=f"w{key}", name=f"w{key}_{rep}_{kg}")
                nc.sync.dma_start(t[:], srcview)
                return t

            def wsum(rep, key, w, kc, m0, m1, width):
                """Karatsuba weight-sum tile on the Pool engine; m0/m1
                are tile-local flat matrix offsets."""
                t = spool.tile([P, width], BF, tag=f"s{key}",
                               name=f"s{key}_{rep}_{kc}")
                nc.gpsimd.tensor_add(
                    t[:], w[:, kc, m0 * width : (m0 + 1) * width],
                    w[:, kc, m1 * width : (m1 + 1) * width])
                return t

            def gate_r(rep, rhTl):
                """Gate r, output-transposed: psum [128 cols, 256 batch].
                Each psum bank holds exactly one accumulation region
                (start=True resets has_written bank-wide). Two cc-pair
                phases per grp; writes rh_re/rh_im/rh12 K-major."""
                g = 1
                for grp in range(2):
                  for ph in range(2):
                    pt = {
                        (X, j): psum.tile([P, BC], F32, tag=f"ps{X}{j}",
                                          name=f"ps{X}{j}_{rep}r{grp}{ph}")
                        for X in "ABC" for j in range(2)
                    }
                    W2 = 2 * P
                    for kg in range(KCH // KG):
                        w = wgroup(rep, "r", WR_r[:, kg, :, grp, ph, :],
                                   kg, 4, W2)
                        for kc in range(KG):
                            k = kg * KG + kc
                            k12 = wsum(rep, "k12", w, kc, 0, 1, W2)
                            r12 = wsum(rep, "r12", w, kc, 2, 3, W2)
                            first, last = k == 0, k == KCH - 1

                            def wv(mi, j):
                                o = mi * W2 + j * P
                                return w[:, kc, o : o + P]

                            for j in range(2):
                                nc.tensor.matmul(
                                    pt[("A", j)][:], wv(0, j),
                                    s["x1"][:, k, :], start=first,
                                    stop=False)
                                nc.tensor.matmul(
                                    pt[("B", j)][:], wv(1, j),
                                    s["x2"][:, k, :], start=first,
                                    stop=False)
                                nc.tensor.matmul(
                                    pt[("A", j)][:], wv(2, j),
                                    s["h1"][:, k, :], start=False,
                                    stop=last)
                                nc.tensor.matmul(
                                    pt[("B", j)][:], wv(3, j),
                                    s["h2"][:, k, :], start=False,
                                    stop=last)
                            for j in range(2):
                                csl = slice(j * P, (j + 1) * P)
                                nc.tensor.matmul(
                                    pt[("C", j)][:], k12[:, csl],
                                    s["x12"][:, k, :], start=first,
                                    stop=False)
                                nc.tensor.matmul(
                                    pt[("C", j)][:], r12[:, csl],
                                    s["h12"][:, k, :], start=False,
                                    stop=last)
                    for j in range(2):
                        cc = ph * 2 + j
                        ccr = grp * 4 + cc          # re k-chunk 0..7
                        cci = 8 + grp * 4 + cc      # im k-chunk 8..15
                        # DVE reads at most one PSUM operand; stage t2
                        # into SBUF via the (idle) scalar engine
                        Bs = small.tile([P, BC], F32, tag="bcr",
                                        name=f"bcr_{rep}_{grp}_{cc}")
                        nc.scalar.activation(
                            Bs[:], pt[("B", j)][:],
                            mybir.ActivationFunctionType.Copy)
                        tre = small.tile([P, BC], F32, tag="rtmp",
                                         name=f"rtmp_{rep}_{grp}_{cc}")
                        nc.vector.tensor_sub(tre[:], pt[("A", j)][:], Bs[:])
                        nc.vector.tensor_scalar(
                            tre[:], tre[:], 0.2, brcol[:, ccr : ccr + 1],
                            mybir.AluOpType.mult, mybir.AluOpType.add,
                        )
                        nc.vector.tensor_scalar(
                            tre[:], tre[:], 1.0, 0.0,
                            mybir.AluOpType.min, mybir.AluOpType.max,
                        )
                        nc.vector.tensor_mul(rhTl[:, ccr, :], tre[:],
                                             s["hTo"][:, ccr, :])
                        tim = small.tile([P, BC], F32, tag="itmp",
                                         name=f"itmp_{rep}_{grp}_{cc}")
                        nc.vector.tensor_add(tim[:], pt[("A", j)][:], Bs[:])
                        nc.vector.tensor_sub(tim[:], pt[("C", j)][:], tim[:])
                        nc.vector.tensor_scalar(
                            tim[:], tim[:], 0.2, brcol[:, cci : cci + 1],
                            mybir.AluOpType.mult, mybir.AluOpType.add,
                        )
                        nc.vector.tensor_scalar(
                            tim[:], tim[:], 1.0, 0.0,
                            mybir.AluOpType.min, mybir.AluOpType.max,
                        )
                        nc.vector.tensor_mul(rhTl[:, cci, :], tim[:],
                                             s["hTo"][:, cci, :])
                        nc.vector.tensor_add(rhTl[:, 16 + ccr, :],
                                             rhTl[:, ccr, :],
                                             rhTl[:, cci, :])

            def bias_bcast(rep, g2, hb, half, tag):
                off = g2 * 2 * UC + half * UC + hb * BLK
                bt = small.tile([P, BLK], F32, tag=tag,
                                name=f"{tag}_{rep}_{g2}_{hb}")
                nc.sync.dma_start(
                    bt[:], bias[None, off : off + BLK].to_broadcast((P, BLK))
                )
                return bt

            def gate_bs(rep, g, g2, a, b, evict, split_xh):
                """Batch-stationary gate (z, h): psum [128 batch, 512 cols]
                t1/t2/t3 banks per m-subtile. a = x-part stationaries
                (uses weight mats 0,1), b = h-part (mats 2,3). split_xh
                runs the whole X sweep before the first H matmul so gate
                h tolerates exchange latency."""
                for hb in range(2):
                    pt = {
                        (X, m): psum.tile([P, BLK], F32, tag=f"ps{X}{m}",
                                          name=f"ps{X}{m}_{rep}_{g}_{hb}")
                        for X in "ABC" for m in range(MSUB)
                    }
                    if split_xh:
                        phases = [((0, 1), a, True), ((2, 3), b, False)]
                    else:
                        phases = [((0, 1, 2, 3), None, None)]
                    for mats, stats, is_x in phases:
                        wkey = "z" if not split_xh else ("hx" if is_x
                                                         else "hh")
                        msl_flat = slice(mats[0] * BLK,
                                         (mats[-1] + 1) * BLK)
                        for kg in range(KCH // KG):
                            w = wgroup(
                                rep, wkey,
                                WZH_r[:, kg, :, g, hb, msl_flat],
                                kg, len(mats), BLK)

                            def wv(mi, kc):
                                return w[:, kc, mi * BLK : (mi + 1) * BLK]

                            for kc in range(KG):
                                k = kg * KG + kc
                                first, last = k == 0, k == KCH - 1
                                if not split_xh:
                                    s12 = wsum(rep, "k12", w, kc, 0, 1, BLK)
                                    h12 = wsum(rep, "r12", w, kc, 2, 3, BLK)
                                    for m in range(MSUB):
                                        msl = slice(m * P, (m + 1) * P)
                                        nc.tensor.matmul(
                                            pt[("A", m)][:], a[0][:, k, msl],
                                            wv(0, kc), start=first,
                                            stop=False)
                                        nc.tensor.matmul(
                                            pt[("B", m)][:], a[1][:, k, msl],
                                            wv(1, kc), start=first,
                                            stop=False)
                                        nc.tensor.matmul(
                                            pt[("A", m)][:], b[0][:, k, msl],
                                            wv(2, kc), start=False,
                                            stop=last)
                                        nc.tensor.matmul(
                                            pt[("B", m)][:], b[1][:, k, msl],
                                            wv(3, kc), start=False,
                                            stop=last)
                                    for m in range(MSUB):
                                        msl = slice(m * P, (m + 1) * P)
                                        nc.tensor.matmul(
                                            pt[("C", m)][:], a[2][:, k, msl],
                                            s12[:], start=first, stop=False)
                                        nc.tensor.matmul(
                                            pt[("C", m)][:], b[2][:, k, msl],
                                            h12[:], start=False, stop=last)
                                else:
                                    s12 = wsum(rep, "k12", w, kc, 0, 1, BLK)
                                    st = first and is_x
                                    sp = last and not is_x
                                    for m in range(MSUB):
                                        msl = slice(m * P, (m + 1) * P)
                                        nc.tensor.matmul(
                                            pt[("A", m)][:],
                                            stats[0][:, k, msl],
                                            wv(0, kc), start=st, stop=sp)
                                        nc.tensor.matmul(
                                            pt[("B", m)][:],
                                            stats[1][:, k, msl],
                                            wv(1, kc), start=st, stop=sp)
                                    for m in range(MSUB):
                                        msl = slice(m * P, (m + 1) * P)
                                        nc.tensor.matmul(
                                            pt[("C", m)][:],
                                            stats[2][:, k, msl],
                                            s12[:], start=st, stop=sp)
                    btre = bias_bcast(rep, g2, hb, 0, "btre")
                    btim = bias_bcast(rep, g2, hb, 1, "btim")
                    for m in range(MSUB):
                        evict(rep, hb, m, pt[("A", m)], pt[("B", m)],
                              pt[("C", m)], btre, btim)

            def evict_z(rep, hb, m, A, Bp, C, btre, btim):
                ocr = slice(hb * BLK, (hb + 1) * BLK)
                oci = slice(UC + hb * BLK, UC + (hb + 1) * BLK)
                Bs = small.tile([P, BLK], F32, tag="bcz",
                                name=f"bcz_{rep}_{hb}_{m}")
                nc.scalar.activation(Bs[:], Bp[:],
                                     mybir.ActivationFunctionType.Copy)
                d = z_sb[:, m, ocr]
                nc.vector.tensor_sub(d, A[:], Bs[:])
                nc.vector.tensor_add(d, d, btre[:])
                nc.vector.tensor_scalar(d, d, 1.0, 0.0,
                                        mybir.AluOpType.min,
                                        mybir.AluOpType.max)
                di = z_sb[:, m, oci]
                tmp = small.tile([P, BLK], F32, tag="ztmp",
                                 name=f"ztmp_{rep}_{hb}_{m}")
                nc.vector.tensor_add(tmp[:], A[:], Bs[:])
                nc.vector.tensor_sub(di, C[:], tmp[:])
                nc.vector.tensor_add(di, di, btim[:])
                nc.vector.tensor_scalar(di, di, 1.0, 0.0,
                                        mybir.AluOpType.min,
                                        mybir.AluOpType.max)

            def evict_h(rep, hb, m, A, Bp, C, btre, btim):
                Bs = small.tile([P, BLK], F32, tag="bch",
                                name=f"bch_{rep}_{hb}_{m}")
                nc.scalar.activation(Bs[:], Bp[:],
                                     mybir.ActivationFunctionType.Copy)
                for half, bt in ((0, btre), (1, btim)):
                    oc = slice(half * UC + hb * BLK,
                               half * UC + (hb + 1) * BLK)
                    tt = small.tile([P, BLK], F32, tag="htmp",
                                    name=f"htmp_{rep}_{hb}_{m}_{half}")
                    if half == 0:
                        nc.vector.tensor_sub(tt[:], A[:], Bs[:])
                    else:
                        nc.vector.tensor_add(tt[:], A[:], Bs[:])
                        nc.vector.tensor_sub(tt[:], C[:], tt[:])
                    nc.vector.tensor_add(tt[:], tt[:], bt[:])
                    hh = small.tile([P, BLK], F32, tag="hh",
                                    name=f"hh_{rep}_{hb}_{m}_{half}")
                    nc.scalar.activation(hh[:], tt[:],
                                         mybir.ActivationFunctionType.Tanh)
                    d = hbmt[:, m, oc]
                    nc.vector.tensor_sub(d, d, hh[:])
                    nc.vector.tensor_mul(d, z_sb[:, m, oc], d)
                    nc.vector.tensor_add(d, hh[:], d)
                    nc.sync.dma_start(out_r[:, m, oc], d)

            for rep in range(repeat):
                # --- gate r (g=1), output-transposed, Karatsuba ---
                rhTl = acts.tile([P, 24, BC], BF, tag="rhTl",
                                 name=f"rhTl_{rep}")
                gate_r(rep, rhTl)
                if rep == 0:
                    # 4MB load deferred here so it doesn't delay the
                    # first gate-r weight tiles; needed only by evict_h
                    nc.sync.dma_start(hbmt[:], hbm_r)

                # pairwise AllGather of (rh_re | rh_im | rh12)
                inb = dram.tile([P, 24, BC], BF, tag="inb",
                                name=f"inb_{rep}")
                outb = dram.tile([2, P, 24, BC], BF, tag="outb",
                                 name=f"outb_{rep}")
                nc.sync.dma_start(inb[:], rhTl[:])
                if sim_collective:
                    # TimelineSim can't model collectives; substitute
                    # bandwidth-equivalent local DMAs
                    nc.sync.dma_start(outb[0], inb[:])
                    nc.sync.dma_start(outb[1], inb[:])
                else:
                    nc.gpsimd.collective_compute(
                        "AllGather",
                        mybir.AluOpType.bypass,
                        replica_groups=[[0, 1], [2, 3], [4, 5], [6, 7]],
                        ins=[inb[:].opt()],
                        outs=[outb[:].opt()],
                    )
                rh1s = acts.tile([P, KCH, BC], BF, tag="rh1s",
                                 name=f"rh1s_{rep}")
                rh2s = acts.tile([P, KCH, BC], BF, tag="rh2s",
                                 name=f"rh2s_{rep}")
                rh12s = acts.tile([P, KCH, BC], BF, tag="rh12s",
                                  name=f"rh12s_{rep}")
                for pi in range(2):
                    ksl = slice(pi * 8, (pi + 1) * 8)
                    nc.sync.dma_start(rh1s[:, ksl, :], outb[pi, :, 0:8, :])
                    nc.sync.dma_start(rh2s[:, ksl, :], outb[pi, :, 8:16, :])
                    nc.sync.dma_start(rh12s[:, ksl, :], outb[pi, :, 16:24, :])

                # --- gate z (g=0), overlaps the collective ---
                gate_bs(rep, 0, 0, (s["x1"], s["x2"], s["x12"]),
                        (s["h1"], s["h2"], s["h12"]), evict_z,
                        split_xh=False)

                # --- gate h (g=2), X-part sweep first, then r*h part ---
                gate_bs(rep, 2, 1, (s["x1"], s["x2"], s["x12"]),
                        (rh1s, rh2s, rh12s), evict_h, split_xh=True)

    nc.compile()
    return nc


def _pack_kmajor(a):
    # (BC, 2048) -> (128, 16, BC) with [p, o, b] = a[b, o*128+p]
    bc = a.shape[0]
    return np.ascontiguousarray(a.T.reshape(KCH, P, bc).transpose(1, 0, 2))


def make_in_maps(
    inputs, h_tm1, real_kernel, imaginary_kernel,
    real_recurrent_kernel, imaginary_recurrent_kernel, real_bias,
    imaginary_bias,
):
    inputs = np.ascontiguousarray(inputs, dtype=np.float32)
    h_tm1 = np.ascontiguousarray(h_tm1, dtype=np.float32)
    rk = np.asarray(real_kernel, np.float32)
    ik = np.asarray(imaginary_kernel, np.float32)
    rr = np.asarray(real_recurrent_kernel, np.float32)
    ir = np.asarray(imaginary_recurrent_kernel, np.float32)
    rb = np.asarray(real_bias, np.float32)
    ib = np.asarray(imaginary_bias, np.float32)

    wsl, bsl, brc = {}, {}, {}
    for p in range(2):
        cols = [slice(g * U + p * UC, g * U + (p + 1) * UC) for g in range(3)]
        scal = [0.2, 1.0, 1.0]  # z folds the hard-sigmoid slope into W

        def wcat(w, sgn):
            return np.concatenate(
                [w[:, c] * (sc * sgn) for c, sc in zip(cols, scal)], axis=1
            ).astype(BF_NP)

        # [2048, mat, 3*1024] packed: k1, k2=-ik, r1, r2=-ir
        wm = np.stack(
            [wcat(rk, 1.0), wcat(ik, -1.0),
             wcat(rr, 1.0), wcat(ir, -1.0)], axis=1
        )
        # WZH [r, gate, hb, (mat c)]; WR [r, grp, ph, (mat c)]
        wzh = np.ascontiguousarray(
            wm.reshape(U, 4, 3, 2, BLK).transpose(0, 2, 3, 1, 4)
            .reshape(U, 3, 2, 4 * BLK)
        )
        wr_ = np.ascontiguousarray(
            wm[:, :, UC : 2 * UC].reshape(U, 4, 2, 2, 2 * P)
            .transpose(0, 2, 3, 1, 4).reshape(U, 2, 2, 4 * 2 * P)
        )
        wsl[p] = (wzh, wr_)
        bz = np.concatenate([rb[cols[0]], ib[cols[0]]])
        bh = np.concatenate([rb[cols[2]], ib[cols[2]]])
        bsl[p] = np.concatenate([0.2 * bz + 0.5, bh]).astype(np.float32)
        br = np.concatenate([rb[cols[1]], ib[cols[1]]])
        brc[p] = np.ascontiguousarray((0.2 * br + 0.5).reshape(KCH, P).T)

    x1f, x2f = inputs[:, :U], inputs[:, U:]
    h1f, h2f = h_tm1[:, :U], h_tm1[:, U:]
    x12f, h12f = x1f + x2f, h1f + h2f

    in_maps = []
    for c in range(N_CORES):
        g, p = c // 2, c % 2
        rs = slice(g * BC, (g + 1) * BC)
        hbm = np.ascontiguousarray(
            np.concatenate(
                [h_tm1[rs, p * UC : (p + 1) * UC],
                 h_tm1[rs, U + p * UC : U + (p + 1) * UC]], axis=1
            )
        )

        def pk(a):
            return _pack_kmajor(a).astype(BF_NP)

        act = np.ascontiguousarray(
            np.stack(
                [pk(x1f[rs]), pk(x2f[rs]), pk(x12f[rs]),
                 pk(h1f[rs]), pk(h2f[rs]), pk(h12f[rs]), pk(hbm)],
                axis=0,
            ).transpose(1, 2, 0, 3).reshape(P, KCH, len(ACT_NAMES) * BC)
        )
        in_maps.append(
            {
                "ACT": act,
                "hbm": hbm,
                "WZH": wsl[p][0],
                "WR": wsl[p][1],
                "bias": bsl[p],
                "biasr": brc[p],
            }
        )
    return in_maps


def scatter_out(results):
    h_new = np.empty((B, 2 * U), dtype=np.float32)
    for c in range(N_CORES):
        g, p = c // 2, c % 2
        rs = slice(g * BC, (g + 1) * BC)
        o = results[c]["out"]
        h_new[rs, p * UC : (p + 1) * UC] = o[:, :UC]
        h_new[rs, U + p * UC : U + (p + 1) * UC] = o[:, UC:]
    return h_new


def _build_nc_retry(repeat=1, attempts=4, **kw):
    # Tile's scheduler very occasionally reports a spurious deadlock on a
    # valid graph (ordering is not fully deterministic); retry a few times.
    last = None
    for _ in range(attempts):
        try:
            return _build_nc(repeat=repeat, **kw)
        except Exception as e:  # noqa: BLE001
            if "Deadlock" not in type(e).__name__ + str(e):
                raise
            last = e
    raise last


def kernel(
    inputs,
    h_tm1,
    real_kernel,
    imaginary_kernel,
    real_recurrent_kernel,
    imaginary_recurrent_kernel,
    real_bias,
    imaginary_bias,
):
    if "nc" not in _CACHE:
        _CACHE["nc"] = _build_nc_retry()
    nc = _CACHE["nc"]
    in_maps = make_in_maps(
        inputs, h_tm1, real_kernel, imaginary_kernel,
        real_recurrent_kernel, imaginary_recurrent_kernel, real_bias,
        imaginary_bias,
    )
    res = run_bass_kernel_spmd(nc, in_maps, core_ids=list(range(N_CORES)))
    return scatter_out(res.results)


# revision 25
# speedup vs baseline: 1.5713x; 1.5713x over previous
"""CGRUCell Trainium2 kernel — hybrid data-parallel x4 (batch) x
tensor-parallel x2 (units) on 8 NeuronCores, with the 3-multiply
Karatsuba/Gauss complex matmul.

The reference's [[Wr,-Wi],[Wi,Wr]] cat is the real form of
(X1 + iX2) @ (Wr - i Wi), i.e. effective W2 = -Wi. Per gate we
accumulate three psum banks over the 4096-long contraction:
  t1 = X1@K1 + H1@R1
  t2 = X2@K2 + H2@R2        (K2 = -imag kernel, host-negated)
  t3 = (X1+X2)@(K1+K2) + (H1+H2)@(R1+R2)
and combine re = t1 - t2, im = t3 - t1 - t2 on the vector engine:
9.7e9 MACs/core instead of 12.9e9 (0.75x; the fp32 roofline was the
baseline's limit). Matmuls run in bf16 (rel err ~5e-3 vs 2e-2 budget):
Karatsuba needs 1.5x the weight bytes and fp32 weights would be
HBM-bound. Only K1,K2,R1,R2 are loaded (50 MB/core); the sum matrices
K12,R12 are formed on-chip by Pool-engine adds.

DMA instruction count is minimized (HWDGE costs ~625ns per DMA,
serialized): the 4 weight matrices are host-packed into one [2048, 4,
3072] tensor and loaded 2 k-chunks x 2 matrices per DMA; the 7
K-major activation tensors are host-packed into one [7,128,16,256]
tensor loaded in 4 chunked DMAs.

Gate r is computed output-transposed (weights stationary) so r*h is
produced directly K-major for gate h's contraction. PSUM start=True
resets has_written for the whole bank, so every accumulation region
gets its own bank: gate r runs two cc-pair phases per column group.
re/im/sum12 of r*h are exchanged with the paired core via AllGather,
hidden under gate z. Gate z's weights are host-prescaled by 0.2 (the
hard-sigmoid slope); gate h evicts through ScalarE tanh and fuses the
final h_new = hh + z*(h - hh) + store.
"""

import sys

for _p in ("/opt/trn_rl_repo", "/root/.axon_site/_ro/trn_rl_repo"):
    if _p not in sys.path:
        sys.path.append(_p)

import numpy as np
import ml_dtypes

import concourse.bass as bass
import concourse.mybir as mybir
import concourse.tile as tile
from concourse import bacc
from concourse.bass_utils import run_bass_kernel_spmd

P = 128
U = 2048           # UNITS
B = 1024
N_CORES = 8
GROUPS = 4
BC = B // GROUPS   # 256 batch rows per core
MSUB = BC // P     # 2 m-subtiles
UC = U // 2        # 1024 complex cols per core
KCH = U // P       # 16 k-chunks per 2048-long half-contraction
KG = 2             # k-chunks loaded per weight DMA
F32 = mybir.dt.float32
BF = mybir.dt.bfloat16
BLK = 512
BF_NP = np.dtype(ml_dtypes.bfloat16)
ACT_NAMES = ("x1", "x2", "x12", "h1", "h2", "h12", "hTo")

_CACHE = {}


def _build_nc(repeat=1, sim_collective=False):
    nc = bacc.Bacc(None, target_bir_lowering=False)

    # packed K-major bf16 activations [part, kchunk, name*batch flat]
    ACT = nc.dram_tensor("ACT", [P, KCH, len(ACT_NAMES) * BC], BF,
                         kind="ExternalInput")
    # batch-major h_tm1, own columns (re|im): [256, 2048] f32
    hbm = nc.dram_tensor("hbm", [BC, 2 * UC], F32, kind="ExternalInput")
    # packed bf16 weights, (mat k1|k2|r1|r2, col) flattened per block so
    # each load is a 3-dim AP; z block pre-scaled 0.2; k2/r2 = -imag
    WZH = nc.dram_tensor("WZH", [U, 3, 2, 4 * BLK], BF,
                         kind="ExternalInput")
    WR = nc.dram_tensor("WR", [U, 2, 2, 4 * 2 * P], BF,
                        kind="ExternalInput")
    # [z'(re|im) | h(re|im)], z' = 0.2*b + 0.5
    bias = nc.dram_tensor("bias", [2 * 2 * UC], F32, kind="ExternalInput")
    # gate-r column-major bias, pre-scaled 0.2*b + 0.5: [128, 16]
    biasr = nc.dram_tensor("biasr", [P, KCH], F32, kind="ExternalInput")
    out = nc.dram_tensor("out", [BC, 2 * UC], F32, kind="ExternalOutput")

    WZH_r = WZH.rearrange("(kg kc p) g hb mc -> p kg kc g hb mc",
                          kc=KG, p=P)
    WR_r = WR.rearrange("(kg kc p) q ph mc -> p kg kc q ph mc",
                        kc=KG, p=P)
    hbm_r = hbm.rearrange("(m p) c -> p m c", p=P)
    out_r = out.rearrange("(m p) c -> p m c", p=P)

    with tile.TileContext(nc) as tc:
        with (
            tc.tile_pool(name="acts", bufs=1) as acts,
            tc.tile_pool(name="wpool", bufs=2) as wpool,
            tc.tile_pool(name="spool", bufs=3) as spool,
            tc.tile_pool(name="psum", bufs=1, space="PSUM") as psum,
            tc.tile_pool(name="small", bufs=2) as small,
            tc.tile_pool(name="bigs", bufs=1) as bigs,
            tc.tile_pool(name="dram", bufs=1, space="DRAM") as dram,
        ):
            A7 = acts.tile([P, KCH, len(ACT_NAMES) * BC], BF, tag="A7",
                           name="A7")
            s = {n: A7[:, :, i * BC : (i + 1) * BC]
                 for i, n in enumerate(ACT_NAMES)}
            # chunked loads so gate-r matmuls start after the first
            # (small) slice; hbmt/brcol are deferred below the gate-r
            # issue so they don't delay the first weight tiles
            for sl in (slice(0, 2), slice(2, 8), slice(8, 16)):
                nc.sync.dma_start(A7[:, sl, :], ACT[:, sl, :])

            brcol = small.tile([P, KCH], F32, tag="brcol", name="brcol",
                               bufs=1)
            nc.sync.dma_start(brcol[:], biasr[:])
            hbmt = bigs.tile([P, MSUB, 2 * UC], F32, tag="hbmt", name="hbmt")
            z_sb = bigs.tile([P, MSUB, 2 * UC], BF, tag="z_sb", name="z_sb")

            def wgroup(rep, key, srcview, kg, nmats, width):
                """One DMA loading [128, KG kchunks, nmats*width flat]
                of packed weights; access mats via flat col offsets."""
                t = wpool.tile([P, KG, nmats * width], BF,
                               tag=f"w{key}", name=f"w{key}_{rep}_{kg}")
                nc.sync.dma_start(t[:], srcview)
                return t

            def wsum(rep, key, w, kc, m0, m1, width):
                """Karatsuba weight-sum tile on DVE (bf16 2x packed
                mode); m0/m1 are tile-local flat matrix offsets."""
                t = spool.tile([P, width], BF, tag=f"s{key}",
                               name=f"s{key}_{rep}_{kc}")
                nc.vector.tensor_add(
                    t[:], w[:, kc, m0 * width : (m0 + 1) * width],
                    w[:, kc, m1 * width : (m1 + 1) * width])
                return t

            def gate_r(rep, rhTl):
                """Gate r, output-transposed: psum [128 cols, 256 batch].
                Each psum bank holds exactly one accumulation region
                (start=True resets has_written bank-wide). Two cc-pair
                phases per grp; writes rh_re/rh_im/rh12 K-major."""
                g = 1
                for grp in range(2):
                  for ph in range(2):
                    pt = {
                        (X, j): psum.tile([P, BC], F32, tag=f"ps{X}{j}",
                                          name=f"ps{X}{j}_{rep}r{grp}{ph}")
                        for X in "ABC" for j in range(2)
                    }
                    W2 = 2 * P
                    for kg in range(KCH // KG):
                        w = wgroup(rep, "r", WR_r[:, kg, :, grp, ph, :],
                                   kg, 4, W2)
                        for kc in range(KG):
                            k = kg * KG + kc
                            k12 = wsum(rep, "k12", w, kc, 0, 1, W2)
                            r12 = wsum(rep, "r12", w, kc, 2, 3, W2)
                            first, last = k == 0, k == KCH - 1

                            def wv(mi, j):
                                o = mi * W2 + j * P
                                return w[:, kc, o : o + P]

                            for j in range(2):
                                nc.tensor.matmul(
                                    pt[("A", j)][:], wv(0, j),
                                    s["x1"][:, k, :], start=first,
                                    stop=False)
                                nc.tensor.matmul(
                                    pt[("B", j)][:], wv(1, j),
                                    s["x2"][:, k, :], start=first,
                                    stop=False)
                                nc.tensor.matmul(
                                    pt[("A", j)][:], wv(2, j),
                                    s["h1"][:, k, :], start=False,
                                    stop=last)
                                nc.tensor.matmul(
                                    pt[("B", j)][:], wv(3, j),
                                    s["h2"][:, k, :], start=False,
                                    stop=last)
                            for j in range(2):
                                csl = slice(j * P, (j + 1) * P)
                                nc.tensor.matmul(
                                    pt[("C", j)][:], k12[:, csl],
                                    s["x12"][:, k, :], start=first,
                                    stop=False)
                                nc.tensor.matmul(
                                    pt[("C", j)][:], r12[:, csl],
                                    s["h12"][:, k, :], start=False,
                                    stop=last)
                    for j in range(2):
                        cc = ph * 2 + j
                        ccr = grp * 4 + cc          # re k-chunk 0..7
                        cci = 8 + grp * 4 + cc      # im k-chunk 8..15
                        # DVE reads at most one PSUM operand; stage t2
                        # into SBUF via the (idle) scalar engine
                        Bs = small.tile([P, BC], F32, tag="bcr",
                                        name=f"bcr_{rep}_{grp}_{cc}")
                        nc.scalar.activation(
                            Bs[:], pt[("B", j)][:],
                            mybir.ActivationFunctionType.Copy)
                        tre = small.tile([P, BC], F32, tag="rtmp",
                                         name=f"rtmp_{rep}_{grp}_{cc}")
                        nc.vector.tensor_sub(tre[:], pt[("A", j)][:], Bs[:])
                        nc.vector.tensor_scalar(
                            tre[:], tre[:], 0.2, brcol[:, ccr : ccr + 1],
                            mybir.AluOpType.mult, mybir.AluOpType.add,
                        )
                        nc.vector.tensor_scalar(
                            tre[:], tre[:], 1.0, 0.0,
                            mybir.AluOpType.min, mybir.AluOpType.max,
                        )
                        nc.vector.tensor_mul(rhTl[:, ccr, :], tre[:],
                                             s["hTo"][:, ccr, :])
                        tim = small.tile([P, BC], F32, tag="itmp",
                                         name=f"itmp_{rep}_{grp}_{cc}")
                        nc.vector.tensor_add(tim[:], pt[("A", j)][:], Bs[:])
                        nc.vector.tensor_sub(tim[:], pt[("C", j)][:], tim[:])
                        nc.vector.tensor_scalar(
                            tim[:], tim[:], 0.2, brcol[:, cci : cci + 1],
                            mybir.AluOpType.mult, mybir.AluOpType.add,
                        )
                        nc.vector.tensor_scalar(
                            tim[:], tim[:], 1.0, 0.0,
                            mybir.AluOpType.min, mybir.AluOpType.max,
                        )
                        nc.vector.tensor_mul(rhTl[:, cci, :], tim[:],
                                             s["hTo"][:, cci, :])
                        nc.vector.tensor_add(rhTl[:, 16 + ccr, :],
                                             rhTl[:, ccr, :],
                                             rhTl[:, cci, :])

            def bias_bcast(rep, g2, hb, half, tag):
                off = g2 * 2 * UC + half * UC + hb * BLK
                bt = small.tile([P, BLK], F32, tag=tag,
                                name=f"{tag}_{rep}_{g2}_{hb}")
                nc.sync.dma_start(
                    bt[:], bias[None, off : off + BLK].to_broadcast((P, BLK))
                )
                return bt

            def gate_bs(rep, g, g2, a, b, evict, split_xh):
                """Batch-stationary gate (z, h): psum [128 batch, 512 cols]
                t1/t2/t3 banks per m-subtile. a = x-part stationaries
                (uses weight mats 0,1), b = h-part (mats 2,3). split_xh
                runs the whole X sweep before the first H matmul so gate
                h tolerates exchange latency."""
                for hb in range(2):
                    pt = {
                        (X, m): psum.tile([P, BLK], F32, tag=f"ps{X}{m}",
                                          name=f"ps{X}{m}_{rep}_{g}_{hb}")
                        for X in "ABC" for m in range(MSUB)
                    }
                    if split_xh:
                        phases = [((0, 1), a, True), ((2, 3), b, False)]
                    else:
                        phases = [((0, 1, 2, 3), None, None)]
                    for mats, stats, is_x in phases:
                        wkey = "z" if not split_xh else ("hx" if is_x
                                                         else "hh")
                        msl_flat = slice(mats[0] * BLK,
                                         (mats[-1] + 1) * BLK)
                        for kg in range(KCH // KG):
                            w = wgroup(
                                rep, wkey,
                                WZH_r[:, kg, :, g, hb, msl_flat],
                                kg, len(mats), BLK)

                            def wv(mi, kc):
                                return w[:, kc, mi * BLK : (mi + 1) * BLK]

                            for kc in range(KG):
                                k = kg * KG + kc
                                first, last = k == 0, k == KCH - 1
                                if not split_xh:
                                    s12 = wsum(rep, "k12", w, kc, 0, 1, BLK)
                                    h12 = wsum(rep, "r12", w, kc, 2, 3, BLK)
                                    for m in range(MSUB):
                                        msl = slice(m * P, (m + 1) * P)
                                        nc.tensor.matmul(
                                            pt[("A", m)][:], a[0][:, k, msl],
                                            wv(0, kc), start=first,
                                            stop=False)
                                        nc.tensor.matmul(
                                            pt[("B", m)][:], a[1][:, k, msl],
                                            wv(1, kc), start=first,
                                            stop=False)
                                        nc.tensor.matmul(
                                            pt[("A", m)][:], b[0][:, k, msl],
                                            wv(2, kc), start=False,
                                            stop=last)
                                        nc.tensor.matmul(
                                            pt[("B", m)][:], b[1][:, k, msl],
                                            wv(3, kc), start=False,
                                            stop=last)
                                    for m in range(MSUB):
                                        msl = slice(m * P, (m + 1) * P)
                                        nc.tensor.matmul(
                                            pt[("C", m)][:], a[2][:, k, msl],
                                            s12[:], start=first, stop=False)
                                        nc.tensor.matmul(
                                            pt[("C", m)][:], b[2][:, k, msl],
                                            h12[:], start=False, stop=last)
                                else:
                                    s12 = wsum(rep, "k12", w, kc, 0, 1, BLK)
                                    st = first and is_x
                                    sp = last and not is_x
                                    for m in range(MSUB):
                                        msl = slice(m * P, (m + 1) * P)
                                        nc.tensor.matmul(
                                            pt[("A", m)][:],
                                            stats[0][:, k, msl],
                                            wv(0, kc), start=st, stop=sp)
                                        nc.tensor.matmul(
                                            pt[("B", m)][:],
                                            stats[1][:, k, msl],
                                            wv(1, kc), start=st, stop=sp)
                                    for m in range(MSUB):
                                        msl = slice(m * P, (m + 1) * P)
                                        nc.tensor.matmul(
                                            pt[("C", m)][:],
                                            stats[2][:, k, msl],
                                            s12[:], start=st, stop=sp)
                    btre = bias_bcast(rep, g2, hb, 0, "btre")
                    btim = bias_bcast(rep, g2, hb, 1, "btim")
                    for m in range(MSUB):
                        evict(rep, hb, m, pt[("A", m)], pt[("B", m)],
                              pt[("C", m)], btre, btim)

            def evict_z(rep, hb, m, A, Bp, C, btre, btim):
                ocr = slice(hb * BLK, (hb + 1) * BLK)
                oci = slice(UC + hb * BLK, UC + (hb + 1) * BLK)
                Bs = small.tile([P, BLK], F32, tag="bcz",
                                name=f"bcz_{rep}_{hb}_{m}")
                nc.scalar.activation(Bs[:], Bp[:],
                                     mybir.ActivationFunctionType.Copy)
                d = z_sb[:, m, ocr]
                nc.vector.tensor_sub(d, A[:], Bs[:])
                nc.vector.tensor_add(d, d, btre[:])
                nc.vector.tensor_scalar(d, d, 1.0, 0.0,
                                        mybir.AluOpType.min,
                                        mybir.AluOpType.max)
                di = z_sb[:, m, oci]
                tmp = small.tile([P, BLK], F32, tag="ztmp",
                                 name=f"ztmp_{rep}_{hb}_{m}")
                nc.vector.tensor_add(tmp[:], A[:], Bs[:])
                nc.vector.tensor_sub(di, C[:], tmp[:])
                nc.vector.tensor_add(di, di, btim[:])
                nc.vector.tensor_scalar(di, di, 1.0, 0.0,
                                        mybir.AluOpType.min,
                                        mybir.AluOpType.max)

            def evict_h(rep, hb, m, A, Bp, C, btre, btim):
                Bs = small.tile([P, BLK], F32, tag="bch",
                                name=f"bch_{rep}_{hb}_{m}")
                nc.scalar.activation(Bs[:], Bp[:],
                                     mybir.ActivationFunctionType.Copy)
                for half, bt in ((0, btre), (1, btim)):
                    oc = slice(half * UC + hb * BLK,
                               half * UC + (hb + 1) * BLK)
                    tt = small.tile([P, BLK], F32, tag="htmp",
                                    name=f"htmp_{rep}_{hb}_{m}_{half}")
                    if half == 0:
                        nc.vector.tensor_sub(tt[:], A[:], Bs[:])
                    else:
                        nc.vector.tensor_add(tt[:], A[:], Bs[:])
                        nc.vector.tensor_sub(tt[:], C[:], tt[:])
                    nc.vector.tensor_add(tt[:], tt[:], bt[:])
                    hh = small.tile([P, BLK], F32, tag="hh",
                                    name=f"hh_{rep}_{hb}_{m}_{half}")
                    nc.scalar.activation(hh[:], tt[:],
                                         mybir.ActivationFunctionType.Tanh)
                    d = hbmt[:, m, oc]
                    nc.vector.tensor_sub(d, d, hh[:])
                    nc.vector.tensor_mul(d, z_sb[:, m, oc], d)
                    nc.vector.tensor_add(d, hh[:], d)
                    nc.sync.dma_start(out_r[:, m, oc], d)

            for rep in range(repeat):
                # --- gate r (g=1), output-transposed, Karatsuba ---
                rhTl = acts.tile([P, 24, BC], BF, tag="rhTl",
                                 name=f"rhTl_{rep}")
                gate_r(rep, rhTl)
                if rep == 0:
                    # 4MB load deferred here so it doesn't delay the
                    # first gate-r weight tiles; needed only by evict_h
                    nc.sync.dma_start(hbmt[:], hbm_r)

                # pairwise AllGather of (rh_re | rh_im | rh12)
                inb = dram.tile([P, 24, BC], BF, tag="inb",
                                name=f"inb_{rep}")
                outb = dram.tile([2, P, 24, BC], BF, tag="outb",
                                 name=f"outb_{rep}")
                nc.sync.dma_start(inb[:], rhTl[:])
                if sim_collective:
                    # TimelineSim can't model collectives; substitute
                    # bandwidth-equivalent local DMAs
                    nc.sync.dma_start(outb[0], inb[:])
                    nc.sync.dma_start(outb[1], inb[:])
                else:
                    nc.gpsimd.collective_compute(
                        "AllGather",
                        mybir.AluOpType.bypass,
                        replica_groups=[[0, 1], [2, 3], [4, 5], [6, 7]],
                        ins=[inb[:].opt()],
                        outs=[outb[:].opt()],
                    )
                rh1s = acts.tile([P, KCH, BC], BF, tag="rh1s",
                                 name=f"rh1s_{rep}")
                rh2s = acts.tile([P, KCH, BC], BF, tag="rh2s",
                                 name=f"rh2s_{rep}")
                rh12s = acts.tile([P, KCH, BC], BF, tag="rh12s",
                                  name=f"rh12s_{rep}")
                for pi in range(2):
                    ksl = slice(pi * 8, (pi + 1) * 8)
                    nc.sync.dma_start(rh1s[:, ksl, :], outb[pi, :, 0:8, :])
                    nc.sync.dma_start(rh2s[:, ksl, :], outb[pi, :, 8:16, :])
                    nc.sync.dma_start(rh12s[:, ksl, :], outb[pi, :, 16:24, :])

                # --- gate z (g=0), overlaps the collective ---
                gate_bs(rep, 0, 0, (s["x1"], s["x2"], s["x12"]),
                        (s["h1"], s["h2"], s["h12"]), evict_z,
                        split_xh=False)

                # --- gate h (g=2), X-part sweep first, then r*h part ---
                gate_bs(rep, 2, 1, (s["x1"], s["x2"], s["x12"]),
                        (rh1s, rh2s, rh12s), evict_h, split_xh=True)

    nc.compile()
    return nc


def _pack_kmajor(a):
    # (BC, 2048) -> (128, 16, BC) with [p, o, b] = a[b, o*128+p]
    bc = a.shape[0]
    return np.ascontiguousarray(a.T.reshape(KCH, P, bc).transpose(1, 0, 2))


def make_in_maps(
    inputs, h_tm1, real_kernel, imaginary_kernel,
    real_recurrent_kernel, imaginary_recurrent_kernel, real_bias,
    imaginary_bias,
):
    inputs = np.ascontiguousarray(inputs, dtype=np.float32)
    h_tm1 = np.ascontiguousarray(h_tm1, dtype=np.float32)
    rk = np.asarray(real_kernel, np.float32)
    ik = np.asarray(imaginary_kernel, np.float32)
    rr = np.asarray(real_recurrent_kernel, np.float32)
    ir = np.asarray(imaginary_recurrent_kernel, np.float32)
    rb = np.asarray(real_bias, np.float32)
    ib = np.asarray(imaginary_bias, np.float32)

    wsl, bsl, brc = {}, {}, {}
    for p in range(2):
        cols = [slice(g * U + p * UC, g * U + (p + 1) * UC) for g in range(3)]
        scal = [0.2, 1.0, 1.0]  # z folds the hard-sigmoid slope into W

        def wcat(w, sgn):
            return np.concatenate(
                [w[:, c] * (sc * sgn) for c, sc in zip(cols, scal)], axis=1
            ).astype(BF_NP)

        # [2048, mat, 3*1024] packed: k1, k2=-ik, r1, r2=-ir
        wm = np.stack(
            [wcat(rk, 1.0), wcat(ik, -1.0),
             wcat(rr, 1.0), wcat(ir, -1.0)], axis=1
        )
        # WZH [r, gate, hb, (mat c)]; WR [r, grp, ph, (mat c)]
        wzh = np.ascontiguousarray(
            wm.reshape(U, 4, 3, 2, BLK).transpose(0, 2, 3, 1, 4)
            .reshape(U, 3, 2, 4 * BLK)
        )
        wr_ = np.ascontiguousarray(
            wm[:, :, UC : 2 * UC].reshape(U, 4, 2, 2, 2 * P)
            .transpose(0, 2, 3, 1, 4).reshape(U, 2, 2, 4 * 2 * P)
        )
        wsl[p] = (wzh, wr_)
        bz = np.concatenate([rb[cols[0]], ib[cols[0]]])
        bh = np.concatenate([rb[cols[2]], ib[cols[2]]])
        bsl[p] = np.concatenate([0.2 * bz + 0.5, bh]).astype(np.float32)
        br = np.concatenate([rb[cols[1]], ib[cols[1]]])
        brc[p] = np.ascontiguousarray((0.2 * br + 0.5).reshape(KCH, P).T)

    x1f, x2f = inputs[:, :U], inputs[:, U:]
    h1f, h2f = h_tm1[:, :U], h_tm1[:, U:]
    x12f, h12f = x1f + x2f, h1f + h2f

    in_maps = []
    for c in range(N_CORES):
        g, p = c // 2, c % 2
        rs = slice(g * BC, (g + 1) * BC)
        hbm = np.ascontiguousarray(
            np.concatenate(
                [h_tm1[rs, p * UC : (p + 1) * UC],
                 h_tm1[rs, U + p * UC : U + (p + 1) * UC]], axis=1
            )
        )

        def pk(a):
            return _pack_kmajor(a).astype(BF_NP)

        act = np.ascontiguousarray(
            np.stack(
                [pk(x1f[rs]), pk(x2f[rs]), pk(x12f[rs]),
                 pk(h1f[rs]), pk(h2f[rs]), pk(h12f[rs]), pk(hbm)],
                axis=0,
            ).transpose(1, 2, 0, 3).reshape(P, KCH, len(ACT_NAMES) * BC)
        )
        in_maps.append(
            {
                "ACT": act,
                "hbm": hbm,
                "WZH": wsl[p][0],
                "WR": wsl[p][1],
                "bias": bsl[p],
                "biasr": brc[p],
            }
        )
    return in_maps


def scatter_out(results):
    h_new = np.empty((B, 2 * U), dtype=np.float32)
    for c in range(N_CORES):
        g, p = c // 2, c % 2
        rs = slice(g * BC, (g + 1) * BC)
        o = results[c]["out"]
        h_new[rs, p * UC : (p + 1) * UC] = o[:, :UC]
        h_new[rs, U + p * UC : U + (p + 1) * UC] = o[:, UC:]
    return h_new


def _build_nc_retry(repeat=1, attempts=4, **kw):
    # Tile's scheduler very occasionally reports a spurious deadlock on a
    # valid graph (ordering is not fully deterministic); retry a few times.
    last = None
    for _ in range(attempts):
        try:
            return _build_nc(repeat=repeat, **kw)
        except Exception as e:  # noqa: BLE001
            if "Deadlock" not in type(e).__name__ + str(e):
                raise
            last = e
    raise last


def kernel(
    inputs,
    h_tm1,
    real_kernel,
    imaginary_kernel,
    real_recurrent_kernel,
    imaginary_recurrent_kernel,
    real_bias,
    imaginary_bias,
):
    if "nc" not in _CACHE:
        _CACHE["nc"] = _build_nc_retry()
    nc = _CACHE["nc"]
    in_maps = make_in_maps(
        inputs, h_tm1, real_kernel, imaginary_kernel,
        real_recurrent_kernel, imaginary_recurrent_kernel, real_bias,
        imaginary_bias,
    )
    res = run_bass_kernel_spmd(nc, in_maps, core_ids=list(range(N_CORES)))
    return scatter_out(res.results)
